# revision 7
# baseline (speedup 1.0000x reference)
"""Trainium2 Bass kernel for nn_MessageLayer (gated message passing GNN).

Strategy (8 NeuronCores, SPMD, no collectives):
  * Host: sort edges by self_fea_idx, split atom ranges across 8 cores with
    ~equal edge counts.  Within a core, edges are packed into "groups" of
    exactly 1024 edges (2 tiles x 512) covering <= 128 distinct consecutive
    atoms; no atom's edges straddle a group.  Pad edges point at an appended
    all-zero atom row (weight 0 => contributes exactly 0).
  * Device (per core): one pass over edge tiles.
      - indirect-DMA gather of self/nbr atom rows (64 feats + weight),
        PE-transpose to feature-major, cast bf16
      - gate & msg MLPs as bf16 matmuls, feature-major [feat, 512]
      - last layer flipped to edge-major [128 edges, cols] so the
        softmax scalars land per-partition
      - segment softmax without segment-max: s = w * exp(gate - G) with a
        host-estimated global G (softmax is shift invariant)
      - scatter-by-segment via a selection-matrix matmul accumulated in PSUM
        over the group; one [128, 195] f32 store per group into an HBM
        accumulator (numer[64]+denom[1] per head)
  * Final pass: indirect-gather each atom's accumulator row, out =
    mean_h(numer_h/denom_h) + atom_in_fea.
"""

import os
import sys

if "/opt/trn_rl_repo" not in sys.path:
    sys.path.insert(0, "/opt/trn_rl_repo")

import numpy as np
import ml_dtypes

import concourse.bass as bass
import concourse.bacc as bacc
import concourse.mybir as mybir
import concourse.tile as tile
from concourse.bass import IndirectOffsetOnAxis, ds

BF16 = ml_dtypes.bfloat16

P = 128          # partitions
E = 512          # edges per tile
CPT = 4          # chunks (of 128 edges) per tile
TPG = 2          # tiles per group
EPG = E * TPG    # edges per group (1024)
NCORES = 8
D = 64           # atom feature len
H = 3            # heads
TBLW = 68        # feat table row: 64 feats + w + 3 pad
ROWW = 65 * H    # accumulator row: (numer 64 + denom 1) per head = 195

AF = mybir.ActivationFunctionType
ALU = mybir.AluOpType
F32 = mybir.dt.float32
BF = mybir.dt.bfloat16
I32 = mybir.dt.int32


# ----------------------------------------------------------------------------
# Host-side preparation
# ----------------------------------------------------------------------------

def _mlp_np(x, Ws, bs):
    for i in range(len(Ws) - 1):
        x = np.maximum(x @ Ws[i] + bs[i], 0.0)
    return x @ Ws[-1] + bs[-1]


def _estimate_G(fea, selfi, nbri, gWs, gbs, n_sample=8192, margin=15.0):
    """Upper-ish bound for gate outputs across all heads (softmax shift)."""
    M = selfi.shape[0]
    rng = np.random.default_rng(12345)
    s = rng.choice(M, size=min(n_sample, M), replace=False)
    fe = np.concatenate([fea[selfi[s]], fea[nbri[s]]], axis=1).astype(np.float32)
    gmax = -np.inf
    for h in range(H):
        g = _mlp_np(fe, [W[h] for W in gWs], [b[h] for b in gbs])
        gmax = max(gmax, float(g.max()))
    return gmax + margin


def _host_prep(atom_weights, atom_in_fea, self_idx, nbr_idx, n_cores=NCORES):
    """Sort/group/pad/swizzle. Returns per-core input arrays + assembly info."""
    N = atom_in_fea.shape[0]
    M = self_idx.shape[0]
    selfi = np.asarray(self_idx).astype(np.int64)
    nbri = np.asarray(nbr_idx).astype(np.int64)

    order = np.argsort(selfi, kind="stable")
    ss = selfi[order]
    sn = nbri[order]

    # core edge boundaries snapped to atom boundaries
    bounds = [0]
    for c in range(1, n_cores):
        pos = c * M // n_cores
        pos = int(np.searchsorted(ss, ss[min(pos, M - 1)], "left"))
        bounds.append(pos)
    bounds.append(M)
    alo = [0] * n_cores
    ahi = [0] * n_cores
    for c in range(n_cores):
        alo[c] = int(ss[bounds[c]]) if bounds[c] < M else N
        if c > 0:
            ahi[c - 1] = alo[c]
    ahi[n_cores - 1] = N
    alo[0] = 0

    # per-core greedy group packing
    cores = []
    for c in range(n_cores):
        e0, e1 = bounds[c], bounds[c + 1]
        na = ahi[c] - alo[c]
        counts = np.bincount((ss[e0:e1] - alo[c]).astype(np.int64), minlength=na)
        groups = []  # (atom_base_local, n_atoms, edge_start, n_edges)
        ga, gn, ge, gm = 0, 0, e0, 0
        for a in range(na):
            cnt = int(counts[a])
            assert cnt <= EPG, "single atom exceeds group capacity"
            if gm + cnt > EPG or gn == P:
                groups.append((ga, gn, ge, gm))
                ga, gn, ge, gm = a, 0, ge + gm, 0
            gn += 1
            gm += cnt
        groups.append((ga, gn, ge, gm))
        assert ge + gm == e1
        cores.append({"alo": alo[c], "ahi": ahi[c], "groups": groups,
                      "e0": e0, "e1": e1})

    n_groups_max = max(len(ci["groups"]) for ci in cores) + 1  # +1 all-pad group
    G_ = n_groups_max
    T = G_ * TPG
    S = G_ * P
    na_max = max(ci["ahi"] - ci["alo"] for ci in cores)
    ATILES = (na_max + P - 1) // P
    APAD = ATILES * P

    ZROW = N  # appended zero atom row index in tbl

    per_core = []
    for c in range(n_cores):
        ci = cores[c]
        gs = ci["groups"]
        esel = np.full((G_, EPG), ZROW, dtype=np.int64)
        enbr = np.full((G_, EPG), ZROW, dtype=np.int64)
        eslot = np.zeros((G_, EPG), dtype=np.int64)
        lin_of_atom = np.full(APAD, G_ * P - P, dtype=np.int64)  # pad group slot 0
        for g, (ga, gn, ge, gm) in enumerate(gs):
            esel[g, :gm] = ss[ge:ge + gm]
            enbr[g, :gm] = sn[ge:ge + gm]
            eslot[g, :gm] = ss[ge:ge + gm] - (ci["alo"] + ga)
            assert eslot[g, :gm].max(initial=0) < P
            a_used = np.unique(ss[ge:ge + gm]) - (ci["alo"] + ga) if gm else []
            lin_of_atom[ga + np.asarray(a_used, dtype=np.int64)] = (
                g * P + np.asarray(a_used, dtype=np.int64))
        # swizzle [G, EPG] -> [G, P, 2, CPT] with e = j*512 + cc*128 + p
        def swz(arr):
            a = arr.reshape(G_, TPG, CPT, P)           # [g, j, cc, p]
            return np.ascontiguousarray(a.transpose(0, 3, 1, 2))  # [g, p, j, cc]

        sself = swz(esel)   # [G, P, 2, 4]
        snbr = swz(enbr)
        sslot = swz(eslot)
        meta = np.empty((G_ * P, 16), dtype=np.int32)
        meta[:, 0:4] = sself[:, :, 0, :].reshape(G_ * P, CPT)
        meta[:, 4:8] = snbr[:, :, 0, :].reshape(G_ * P, CPT)
        meta[:, 8:12] = sself[:, :, 1, :].reshape(G_ * P, CPT)
        meta[:, 12:16] = snbr[:, :, 1, :].reshape(G_ * P, CPT)
        slotv = np.empty((G_ * P, 2 * CPT), dtype=BF16)
        slotv[:, 0:4] = sslot[:, :, 0, :].reshape(G_ * P, CPT).astype(BF16)
        slotv[:, 4:8] = sslot[:, :, 1, :].reshape(G_ * P, CPT).astype(BF16)

        pa = lin_of_atom.reshape(APAD, 1).astype(np.int32)

        afea = np.zeros((APAD, D), dtype=np.float32)
        na = ci["ahi"] - ci["alo"]
        afea[:na] = atom_in_fea[ci["alo"]:ci["ahi"]]
        per_core.append({"meta": meta, "slotv": slotv, "pa": pa,
                         "afea": afea.reshape(ATILES, P, D),
                         "alo": ci["alo"], "na": na})

    tbl = np.zeros((N + 1, TBLW), dtype=np.float32)
    tbl[:N, :D] = atom_in_fea
    tbl[:N, D] = np.asarray(atom_weights).reshape(-1)

    return {"per_core": per_core, "tbl": tbl, "G": G_, "T": T, "S": S,
            "ATILES": ATILES, "APAD": APAD, "N": N}


def _pack_weights(gate_params, msg_params, Gmax):
    """Pack weights into kernel layouts.  combo c = h*2 + m (m=0 gate, 1 msg)."""
    gW = [np.asarray(W, np.float32) for W, b in gate_params]
    gb = [np.asarray(b, np.float32) for W, b in gate_params]
    mW = [np.asarray(W, np.float32) for W, b in msg_params]
    mb = [np.asarray(b, np.float32) for W, b in msg_params]

    def Wof(c, li):
        h, m = divmod(c, 2)
        return (gW, mW)[m][li][h]

    def bof(c, li):
        h, m = divmod(c, 2)
        return (gb, mb)[m][li][h]

    w1 = np.zeros((P, 6 * 384), dtype=BF16)
    b1 = np.zeros((P, 6 * 3), dtype=np.float32)
    for c in range(6):
        w1[:, c * 384:(c + 1) * 384] = Wof(c, 0).astype(BF16)
        for j in range(3):
            b1[:, c * 3 + j] = bof(c, 0)[j * P:(j + 1) * P]

    # w2 cols: ((c*3 + kc)*2 + j2)*128
    w2 = np.zeros((P, 6 * 3 * 2 * P), dtype=BF16)
    b2 = np.zeros((P, 6 * 2), dtype=np.float32)
    for c in range(6):
        W = Wof(c, 1)  # [384, 256]
        for kc in range(3):
            for j2 in range(2):
                col = ((c * 3 + kc) * 2 + j2) * P
                w2[:, col:col + P] = W[kc * P:(kc + 1) * P,
                                       j2 * P:(j2 + 1) * P].astype(BF16)
        for j2 in range(2):
            b2[:, c * 2 + j2] = bof(c, 1)[j2 * P:(j2 + 1) * P]

    # L3: gate [256 -> 64], msg [256 -> 128]; 2 K-chunks each
    w3g = np.zeros((P, 3 * 2 * 64), dtype=BF16)
    w3m = np.zeros((P, 3 * 2 * P), dtype=BF16)
    b3g = np.zeros((P, 3), dtype=np.float32)
    b3m = np.zeros((P, 3), dtype=np.float32)
    for h in range(3):
        Wg = gW[2][h]  # [256, 64]
        Wm = mW[2][h]  # [256, 128]
        for kc in range(2):
            w3g[:, (h * 2 + kc) * 64:(h * 2 + kc + 1) * 64] = \
                Wg[kc * P:(kc + 1) * P].astype(BF16)
            w3m[:, (h * 2 + kc) * P:(h * 2 + kc + 1) * P] = \
                Wm[kc * P:(kc + 1) * P].astype(BF16)
        b3g[:64, h] = gb[2][h]
        b3m[:, h] = mb[2][h]

    # L4 edge-major rhs: per head 65 cols [msg 64 | gate 1]
    w4m = np.zeros((P, 3 * 65), dtype=BF16)       # K=128 (msg x3)
    w4g = np.zeros((65, 3 * 65), dtype=BF16)      # K=65 (gate x3 64 + ones row)
    cbias = np.zeros((P, 3), dtype=np.float32)    # b4gate - G (exp bias)
    for h in range(3):
        w4m[:, h * 65:h * 65 + 64] = mW[3][h].astype(BF16)        # [128,64]
        w4g[0:64, h * 65 + 64] = gW[3][h][:, 0].astype(BF16)      # [64,1]
        w4g[64, h * 65:h * 65 + 64] = mb[3][h].astype(BF16)       # msg bias row
        cbias[:, h] = float(gb[3][h][0]) - Gmax

    iota = np.broadcast_to(np.arange(P, dtype=np.float32), (P, P)).astype(BF16)
    iota = np.ascontiguousarray(iota)
    ident = np.eye(P, dtype=np.float32)

    return {"w1": w1, "b1": b1, "w2": w2, "b2": b2, "w3g": w3g, "w3m": w3m,
            "b3g": b3g, "b3m": b3m, "w4m": w4m, "w4g": w4g, "cbias": cbias,
            "iota": iota, "ident": ident}


# ----------------------------------------------------------------------------
# Bass kernel builder
# ----------------------------------------------------------------------------

def build_bass(G_, ATILES, Ntbl, num_devices=NCORES, unroll_groups=False,
               evac_split=True):
    """Build the Bass module.  Identical across cores (SPMD)."""
    T = G_ * TPG
    S = G_ * P

    nc = bacc.Bacc("TRN2", target_bir_lowering=False, debug=False,
                   enable_asserts=False, num_devices=num_devices)

    tbl = nc.dram_tensor("tbl", [Ntbl, TBLW], F32, kind="ExternalInput").ap()
    meta = nc.dram_tensor("meta", [G_ * P, 16], I32, kind="ExternalInput").ap()
    slotv = nc.dram_tensor("slotv", [G_ * P, 2 * CPT], BF, kind="ExternalInput").ap()
    pa = nc.dram_tensor("pa", [ATILES * P, 1], I32, kind="ExternalInput").ap()
    afea = nc.dram_tensor("afea", [ATILES, P, D], F32, kind="ExternalInput").ap()

    wnames = {"w1": (P, 6 * 384, BF), "b1": (P, 18, F32),
              "w2": (P, 4608, BF), "b2": (P, 12, F32),
              "w3g": (P, 384, BF), "w3m": (P, 768, BF),
              "b3g": (P, 3, F32), "b3m": (P, 3, F32),
              "w4m": (P, 195, BF), "w4g": (65, 195, BF),
              "cbias": (P, 3, F32), "iota": (P, P, BF), "ident": (P, P, F32)}
    wdram = {k: nc.dram_tensor(k, [p_, f_], dt, kind="ExternalInput").ap()
             for k, (p_, f_, dt) in wnames.items()}

    acc = nc.dram_tensor("acc_lin", [S, ROWW], F32).ap()
    outp = nc.dram_tensor("outp", [ATILES, P, D], F32, kind="ExternalOutput").ap()

    with tile.TileContext(nc) as tc:
        with (
            tc.tile_pool(name="const", bufs=1) as cpool,
            tc.tile_pool(name="work", bufs=2) as wpool,
            tc.tile_pool(name="x3", bufs=2) as x3pool,
            tc.tile_pool(name="psum", bufs=2, space="PSUM") as ppool,
        ):
            # --- load constants into SBUF
            W = {}
            for k, (p_, f_, dt) in wnames.items():
                t = cpool.tile([p_, f_], dt, tag=f"c_{k}")
                nc.sync.dma_start(t[:, :], wdram[k][:, :])
                W[k] = t

            def emit_tile(meta_t, slot_t, goff, j, first, psum_g):
                """One 512-edge tile; j = tile index within group."""
                # gathers (edge-major [128, 4*68] f32)
                gself = wpool.tile([P, CPT * TBLW], F32, tag="gself")
                gnbr = wpool.tile([P, CPT * TBLW], F32, tag="gnbr")
                for cc in range(CPT):
                    nc.gpsimd.indirect_dma_start(
                        out=gself[:, cc * TBLW:(cc + 1) * TBLW], out_offset=None,
                        in_=tbl[:, :],
                        in_offset=IndirectOffsetOnAxis(
                            ap=meta_t[:, j * 8 + cc:j * 8 + cc + 1], axis=0))
                    nc.gpsimd.indirect_dma_start(
                        out=gnbr[:, cc * TBLW:(cc + 1) * TBLW], out_offset=None,
                        in_=tbl[:, :],
                        in_offset=IndirectOffsetOnAxis(
                            ap=meta_t[:, j * 8 + 4 + cc:j * 8 + 4 + cc + 1], axis=0))

                # transpose to feature-major bf16 [128, 512]
                feaT = wpool.tile([P, E], BF, tag="feaT")
                for cc in range(CPT):
                    pt = ppool.tile([D, P], F32, tag="ptr")
                    nc.tensor.transpose(pt[:, :], gself[:, cc * TBLW:cc * TBLW + D],
                                        W["ident"][:, :])
                    if cc % 2 == 0:
                        nc.scalar.copy(feaT[0:D, cc * P:(cc + 1) * P], pt[:, :])
                    else:
                        nc.vector.tensor_copy(feaT[0:D, cc * P:(cc + 1) * P], pt[:, :])
                    pt2 = ppool.tile([D, P], F32, tag="ptr")
                    nc.tensor.transpose(pt2[:, :], gnbr[:, cc * TBLW:cc * TBLW + D],
                                        W["ident"][:, :])
                    if cc % 2 == 1:
                        nc.scalar.copy(feaT[D:P, cc * P:(cc + 1) * P], pt2[:, :])
                    else:
                        nc.vector.tensor_copy(feaT[D:P, cc * P:(cc + 1) * P], pt2[:, :])

                # --- MLPs feature-major, all 6 combos; keep x3 per head
                x3g = [None] * 3
                x3m = [None] * 3
                evac_i = 0
                for c in range(6):
                    h, m = divmod(c, 2)
                    x1 = wpool.tile([P, 3 * E], BF, tag="x1")
                    for jc in range(3):
                        ps = ppool.tile([P, E], F32, tag="mlp")
                        nc.tensor.matmul(ps[:, :],
                                         lhsT=W["w1"][:, c * 384 + jc * P:c * 384 + (jc + 1) * P],
                                         rhs=feaT[:, :], start=True, stop=True)
                        dst = x1[:, jc * E:(jc + 1) * E]
                        if evac_split and evac_i % 2 == 0:
                            nc.scalar.activation(dst, ps[:, :], AF.Relu,
                                                 bias=W["b1"][:, c * 3 + jc:c * 3 + jc + 1])
                        else:
                            nc.vector.tensor_scalar(
                                dst, ps[:, :], W["b1"][:, c * 3 + jc:c * 3 + jc + 1],
                                0.0, ALU.add, ALU.max)
                        evac_i += 1
                    x2 = wpool.tile([P, 2 * E], BF, tag="x2")
                    for j2 in range(2):
                        ps = ppool.tile([P, E], F32, tag="mlp")
                        for kc in range(3):
                            col = ((c * 3 + kc) * 2 + j2) * P
                            nc.tensor.matmul(ps[:, :], lhsT=W["w2"][:, col:col + P],
                                             rhs=x1[:, kc * E:(kc + 1) * E],
                                             start=(kc == 0), stop=(kc == 2))
                        dst = x2[:, j2 * E:(j2 + 1) * E]
                        if evac_split and evac_i % 2 == 0:
                            nc.scalar.activation(dst, ps[:, :], AF.Relu,
                                                 bias=W["b2"][:, c * 2 + j2:c * 2 + j2 + 1])
                        else:
                            nc.vector.tensor_scalar(
                                dst, ps[:, :], W["b2"][:, c * 2 + j2:c * 2 + j2 + 1],
                                0.0, ALU.add, ALU.max)
                        evac_i += 1
                    if m == 0:
                        x3 = x3pool.tile([65, E], BF, tag=f"x3g{h}")
                        ps = ppool.tile([64, E], F32, tag="mlp")
                        for kc in range(2):
                            nc.tensor.matmul(ps[:, :],
                                             lhsT=W["w3g"][:, (h * 2 + kc) * 64:(h * 2 + kc + 1) * 64],
                                             rhs=x2[:, kc * E:(kc + 1) * E],
                                             start=(kc == 0), stop=(kc == 1))
                        nc.scalar.activation(x3[0:64, :], ps[:, :], AF.Relu,
                                             bias=W["b3g"][0:64, h:h + 1])
                        nc.gpsimd.memset(x3[64:65, :], 1.0)
                        x3g[h] = x3
                    else:
                        x3 = x3pool.tile([P, E], BF, tag=f"x3m{h}")
                        ps = ppool.tile([P, E], F32, tag="mlp")
                        for kc in range(2):
                            nc.tensor.matmul(ps[:, :],
                                             lhsT=W["w3m"][:, (h * 2 + kc) * P:(h * 2 + kc + 1) * P],
                                             rhs=x2[:, kc * E:(kc + 1) * E],
                                             start=(kc == 0), stop=(kc == 1))
                        nc.vector.tensor_scalar(x3[:, :], ps[:, :],
                                                W["b3m"][:, h:h + 1], 0.0,
                                                ALU.add, ALU.max)
                        x3m[h] = x3

                # --- per chunk: L4 flip to edge-major, softmax, scatter
                for cc in range(CPT):
                    pse = ppool.tile([P, ROWW], F32, tag="pse")
                    for h in range(3):
                        nc.tensor.matmul(pse[:, h * 65:(h + 1) * 65],
                                         lhsT=x3m[h][:, cc * P:(cc + 1) * P],
                                         rhs=W["w4m"][:, h * 65:(h + 1) * 65],
                                         start=(h == 0), stop=False)
                        nc.tensor.matmul(pse[:, h * 65:(h + 1) * 65],
                                         lhsT=x3g[h][:, cc * P:(cc + 1) * P],
                                         rhs=W["w4g"][:, h * 65:(h + 1) * 65],
                                         start=False, stop=(h == 2))
                    exps = wpool.tile([P, 3], F32, tag="exps")
                    for h in range(3):
                        nc.scalar.activation(exps[:, h:h + 1],
                                             pse[:, h * 65 + 64:h * 65 + 65],
                                             AF.Exp, bias=W["cbias"][:, h:h + 1])
                    s_t = wpool.tile([P, 3], F32, tag="s_t")
                    nc.vector.tensor_scalar_mul(
                        s_t[:, :], exps[:, :],
                        gnbr[:, cc * TBLW + D:cc * TBLW + D + 1])
                    sv = wpool.tile([P, ROWW], BF, tag="sv")
                    for h in range(3):
                        nc.vector.tensor_scalar_mul(sv[:, h * 65:h * 65 + 64],
                                                    pse[:, h * 65:h * 65 + 64],
                                                    s_t[:, h:h + 1])
                        nc.scalar.copy(sv[:, h * 65 + 64:h * 65 + 65],
                                       s_t[:, h:h + 1])
                    sel = wpool.tile([P, P], BF, tag="sel")
                    nc.vector.tensor_tensor(
                        sel[:, :],
                        in0=slot_t[:, j * CPT + cc:j * CPT + cc + 1].to_broadcast([P, P]),
                        in1=W["iota"][:, :], op=ALU.is_equal)
                    for h in range(3):
                        fst = first and (cc == 0) and (h == 0)
                        lst = (j == TPG - 1) and (cc == CPT - 1) and (h == 2)
                        nc.tensor.matmul(psum_g[:, h * 65:(h + 1) * 65],
                                         lhsT=sel[:, :], rhs=sv[:, h * 65:(h + 1) * 65],
                                         start=fst, stop=lst)

            def emit_group(goff):
                meta_t = wpool.tile([P, 16], I32, tag="meta")
                nc.sync.dma_start(meta_t[:, :], meta[ds(goff, P), :])
                slot_t = wpool.tile([P, 2 * CPT], BF, tag="slot")
                nc.sync.dma_start(slot_t[:, :], slotv[ds(goff, P), :])
                psum_g = ppool.tile([P, ROWW], F32, tag="pacc")
                for j in range(TPG):
                    emit_tile(meta_t, slot_t, goff, j, first=(j == 0), psum_g=psum_g)
                stage = wpool.tile([P, ROWW], F32, tag="stage")
                nc.vector.tensor_copy(stage[:, :], psum_g[:, :])
                nc.sync.dma_start(acc[ds(goff, P), :], stage[:, :])

            if unroll_groups:
                for g in range(G_):
                    emit_group(g * P)
            else:
                with tc.For_i(0, G_ * P, P) as goff:
                    emit_group(goff)

            # ---------------- final per-atom pass ----------------
            for at in range(ATILES):
                pat = wpool.tile([P, 1], I32, tag="pat")
                nc.sync.dma_start(pat[:, :], pa[at * P:(at + 1) * P, :])
                gacc = wpool.tile([P, ROWW], F32, tag="gacc")
                nc.gpsimd.indirect_dma_start(
                    out=gacc[:, :], out_offset=None, in_=acc[:, :],
                    in_offset=IndirectOffsetOnAxis(ap=pat[:, 0:1], axis=0))
                af_t = wpool.tile([P, D], F32, tag="af")
                nc.sync.dma_start(af_t[:, :], afea[at, :, :])
                dn = wpool.tile([P, 3], F32, tag="dn")
                nc.vector.tensor_scalar_add(dn[:, :], gacc[:, 64:ROWW:65], 1e-30)
                rd = wpool.tile([P, 3], F32, tag="rd")
                nc.vector.reciprocal(rd[:, :], dn[:, :])
                acc_t = wpool.tile([P, D], F32, tag="acct")
                tmp_t = wpool.tile([P, D], F32, tag="tmpt")
                nc.vector.tensor_scalar_mul(acc_t[:, :], gacc[:, 0:64], rd[:, 0:1])
                for h in (1, 2):
                    nc.vector.tensor_scalar_mul(tmp_t[:, :],
                                                gacc[:, h * 65:h * 65 + 64],
                                                rd[:, h:h + 1])
                    nc.vector.tensor_tensor(acc_t[:, :], in0=acc_t[:, :],
                                            in1=tmp_t[:, :], op=ALU.add)
                out_t = wpool.tile([P, D], F32, tag="outt")
                nc.vector.tensor_scalar(out_t[:, :], acc_t[:, :],
                                        1.0 / 3.0, None, ALU.mult)
                nc.vector.tensor_tensor(out_t[:, :], in0=out_t[:, :],
                                        in1=af_t[:, :], op=ALU.add)
                nc.sync.dma_start(outp[at, :, :], out_t[:, :])

    nc.compile()
    return nc


# ----------------------------------------------------------------------------
# Entry point
# ----------------------------------------------------------------------------

def _prep_all(inputs):
    atom_weights = np.asarray(inputs["atom_weights"], dtype=np.float32)
    atom_in_fea = np.asarray(inputs["atom_in_fea"], dtype=np.float32)
    self_idx = np.asarray(inputs["self_fea_idx"])
    nbr_idx = np.asarray(inputs["nbr_fea_idx"])
    gate_params = inputs["gate_params"]
    msg_params = inputs["msg_params"]

    gWs = [np.asarray(W, np.float32) for W, b in gate_params]
    gbs = [np.asarray(b, np.float32) for W, b in gate_params]
    fea_bf = atom_in_fea.astype(BF16).astype(np.float32)
    Gmax = _estimate_G(fea_bf, self_idx.astype(np.int64),
                       nbr_idx.astype(np.int64), gWs, gbs)

    hp = _host_prep(atom_weights, atom_in_fea, self_idx, nbr_idx)
    wp = _pack_weights(gate_params, msg_params, Gmax)

    in_maps = []
    for c in range(NCORES):
        pc = hp["per_core"][c]
        im = {"tbl": hp["tbl"], "meta": pc["meta"], "slotv": pc["slotv"],
              "pa": pc["pa"], "afea": pc["afea"]}
        im.update({k: np.ascontiguousarray(v) for k, v in wp.items()})
        in_maps.append(im)
    return hp, in_maps


def kernel(**inputs) -> np.ndarray:
    from concourse.bass_utils import run_bass_kernel_spmd

    hp, in_maps = _prep_all(inputs)
    nc = build_bass(hp["G"], hp["ATILES"], hp["N"] + 1,
                    unroll_groups=os.environ.get("K_UNROLL", "0") == "1")

    res = run_bass_kernel_spmd(
        nc, in_maps, core_ids=list(range(NCORES)),
        trace=os.environ.get("K_TRACE", "0") == "1")

    N = hp["N"]
    out = np.zeros((N, D), dtype=np.float32)
    for c in range(NCORES):
        pc = hp["per_core"][c]
        o = res.results[c]["outp"].reshape(hp["APAD"], D)
        out[pc["alo"]:pc["alo"] + pc["na"]] = o[:pc["na"]]
    kernel._last_results = res
    return out
